# revision 38
# baseline (speedup 1.0000x reference)
"""3-layer GCN (PyG GCNConv-style) on 8 Trainium2 NeuronCores — v3.

Strategy (graph/data parallel; nodes sharded by destination core):
  - Nodes partitioned contiguously: 6272 per core (49 windows x 128). Edges
    (incl. host-added self-loops) are owned by the core owning their dst.
  - Normalization is folded away: the gathered xw table holds
    dinv[src] * (h @ W) rows, and the window epilogue applies the dinv[dst]
    factor as the per-partition `scale` of the sigmoid activation. The bias
    is injected into PSUM via a K=1 rank-1 matmul with a sqrt(deg[dst])
    column so it survives the later dinv[dst] scaling.
  - The per-edge one-hot scatter matrices are graph-static: built ONCE on
    the host in fp8 (exact for 0/1) and streamed from DRAM each layer.
  - Gathers use int16 indices, so the 50176-row xw table is split into
    half-tables A (local row < 3200, 25600 rows) and B (24576 rows). Each
    layer runs two phases: phase A processes every window's A-half tiles
    (window-major, parked to SBUF in bf16 at window close), phase B
    re-injects the parked sum via an identity matmul and finishes the
    window (sigmoid epilogue, agent tap, PE transpose into the next
    layer's hT). Phases are split into CMAX-tile gather chunks, each
    issued as 8-tile sub-gathers (single_packet coalescing caps a call at
    64 descs/engine) spread round-robin over the 4 SWDGE queues; the
    steady state is bound by SDMA descriptor execution (~45ns per 256B
    descriptor per engine across 16 engines).
  - v3 over v2:
    * Layer 2 uses a second, smaller edge schedule keeping only edges into
      tapped destinations (agents, dst%4==0): 875 -> 294 gather tiles.
    * Layer 0's gather tables and slot-major xw stage are computed on the
      HOST (x and W0 are known), removing both layer-0 AllGathers and the
      ~110us serial startup they caused.
    * AllGather DRAM buffers ping-pong (bufs=2) so a layer's collective
      never waits for the previous layer's gather reads to drain (this
      WAR stall exposed ~120us per layer on the tail).
    * Each window's epilogue immediately computes the NEXT layer's xw
      column for that window (matmul + dinv evac) and stores it straight
      into this core's slice of the AllGather input buffer. Every AG
      trigger therefore finds its input already in DRAM: AG_b(l) fires
      after the first phase-A run and overlaps the whole phase-A gather
      stream; AG_a(l+1) fires right after the run containing the last
      lo-window epilogue and overlaps the phase-B tail. Previously the
      triggers waited 20-64us on a PE compute chain, stalling the gpsimd
      gather stream behind them.

Host-side work: graph preprocessing (degrees, edge layout, one-hot tiles,
layer-0 tables) and final output assembly.
"""

import sys

sys.path.insert(0, "/opt/trn_rl_repo")

import numpy as np
import ml_dtypes

N_NODES = 50000
D = 64
N_CORES = 8
WSZ = 128               # dst-window size (PSUM partition dim)
NW = 49                 # windows per core
NPC = NW * WSZ          # 6272 padded nodes per core (50176 total >= 50000)
HALF_A = 3200           # local rows < HALF_A -> table A (25 windows' rows)
HALF_B = NPC - HALF_A   # 3072 rows -> table B
ROWS_A = N_CORES * HALF_A   # 25600 (< 32767, int16-addressable)
ROWS_B = N_CORES * HALF_B   # 24576
CMAX = 48               # max tiles per gather chunk; small chunks + deep
                        # buffering let 4 queues (= Q7 core pairs) gen in parallel


def _layout(s_all, d_all):
    """Edge layout + one-hot scatter tiles for one edge set.

    Tile stream: [phase A: w0..w48, each window's A-half tiles]
                 [phase B: w0..w48, each window's B-half tiles].
    SPMD: tile counts per (window, half) are maxed over cores; padded slots
    get all-zero one-hot rows so they contribute nothing.
    """
    core = d_all // NPC
    local = d_all - core * NPC
    win = local // WSZ
    col = local % WSZ

    s_core = s_all // NPC
    s_loc = s_all - s_core * NPC
    half = (s_loc >= HALF_A).astype(np.int64)
    idx16 = np.where(half == 0, s_core * HALF_A + s_loc,
                     s_core * HALF_B + (s_loc - HALF_A))

    # group edges by (core, half, win)
    key = (core * 2 + half) * NW + win
    nkey = N_CORES * 2 * NW
    order = np.argsort(key, kind="stable")
    key_sorted = key[order]
    bounds = np.searchsorted(key_sorted, np.arange(nkey + 1))
    cnt = (bounds[1:] - bounds[:-1]).reshape(N_CORES, 2, NW)

    # uniform tiles per (half, win), maxed over cores
    n_th = -(-cnt.max(axis=0) // WSZ)               # [2, NW]
    T = int(n_th.sum())

    # tile stream + gather chunks (runs)
    tile_win = []
    runs = []                                       # (t0, nt, half)
    win_tile_base = np.zeros((2, NW), np.int64)
    for h in (0, 1):
        p0 = len(tile_win)
        for w in range(NW):
            win_tile_base[h, w] = len(tile_win)
            tile_win += [w] * int(n_th[h, w])
        np_h = len(tile_win) - p0                   # tiles in this phase
        if np_h == 0:
            continue
        n_chunks = -(-np_h // CMAX)
        splits = np.linspace(p0, p0 + np_h, n_chunks + 1).astype(np.int64)
        for a, b in zip(splits[:-1], splits[1:]):
            if b > a:
                runs.append((int(a), int(b - a), h))
    tile_win = np.asarray(tile_win)
    assert len(tile_win) == T
    max_run = max(nt for _, nt, _ in runs)

    # per-window first/last tile within each phase (-1 if none)
    wfirst = np.full((2, NW), -1, np.int64)
    wlast = np.full((2, NW), -1, np.int64)
    for h in (0, 1):
        for w in range(NW):
            if n_th[h, w] > 0:
                wfirst[h, w] = win_tile_base[h, w]
                wlast[h, w] = win_tile_base[h, w] + n_th[h, w] - 1

    # per-core edge slot arrays
    idx_flat = np.zeros((N_CORES, T * WSZ), np.int16)
    vm8 = np.zeros((N_CORES, WSZ, T * WSZ), ml_dtypes.float8_e4m3)
    for c in range(N_CORES):
        for h in (0, 1):
            for w in range(NW):
                gidx = (c * 2 + h) * NW + w
                e0, e1 = bounds[gidx], bounds[gidx + 1]
                n = e1 - e0
                if n == 0:
                    continue
                sel = order[e0:e1]
                base = win_tile_base[h, w] * WSZ
                pos = base + np.arange(n)
                idx_flat[c, pos] = idx16[sel].astype(np.int16)
                tt = pos // WSZ
                pp = pos % WSZ
                vm8[c, pp, tt * WSZ + col[sel]] = 1.0

    # wrap indices for dma_gather: [128, T*8] int16,
    # arr[p, t*8 + cc] = idx[t*128 + cc*16 + (p % 16)]
    w16 = idx_flat.reshape(N_CORES, T, 8, 16).transpose(0, 3, 1, 2).reshape(
        N_CORES, 16, T * 8)
    idx_arr = np.tile(w16, (1, 8, 1))               # [N_CORES, 128, T*8]

    sched = dict(T=T, runs=runs, tile_win=tile_win, n_th=n_th,
                 wfirst=wfirst, wlast=wlast, max_run=max_run)
    return idx_arr, vm8, sched


def _preprocess(edge_index):
    src = np.asarray(edge_index[0], dtype=np.int64)
    dst = np.asarray(edge_index[1], dtype=np.int64)

    deg = np.bincount(dst, minlength=N_NODES).astype(np.float32) + 1.0
    dinv = (1.0 / np.sqrt(deg)).astype(np.float32)
    sqdeg = np.sqrt(deg).astype(np.float32)

    # full edge set (layers 0, 1); self-loops are NOT gathered: their
    # dinv[i]*xw[i] rows live in the local xw_stage (identity matmul)
    idx_f, vm_f, sched_f = _layout(src, dst)
    # layer 2 only needs destinations that are tapped: agent rows (dst%4==0)
    m2 = (dst % 4) == 0
    idx_2, vm_2, sched_2 = _layout(src[m2], dst[m2])

    dinv_pad = np.ones(N_CORES * NPC, np.float32)
    sqdeg_pad = np.ones(N_CORES * NPC, np.float32)
    dinv_pad[:N_NODES] = dinv
    sqdeg_pad[:N_NODES] = sqdeg
    dinv_own = dinv_pad.reshape(N_CORES, NW, WSZ).transpose(0, 2, 1).copy()
    sqdeg_own = sqdeg_pad.reshape(N_CORES, 1, NPC).astype(ml_dtypes.bfloat16)

    return (idx_f, vm_f, sched_f), (idx_2, vm_2, sched_2), \
        dinv_own, sqdeg_own, dinv_pad


def _build_program(sched_f, sched_2):
    import os
    VAR = set(os.environ.get("KVAR", "").split(","))
    import concourse.bass as bass
    import concourse.bacc as bacc
    import concourse.tile as tile
    from concourse import mybir
    from concourse.bass import BassGpSimd

    f32 = mybir.dt.float32
    bf16 = mybir.dt.bfloat16
    fp8 = mybir.dt.float8e4
    i16 = mybir.dt.int16

    scheds = [sched_f, sched_f, sched_2]
    Tf = sched_f["T"]
    T2 = sched_2["T"]
    max_run = max(sched_f["max_run"], sched_2["max_run"])

    nsq = 4
    nc = bacc.Bacc("TRN2", target_bir_lowering=False, debug=False,
                   num_devices=N_CORES, num_swdge_queues=nsq)

    xT_own = nc.dram_tensor("xT_own", [64, NPC], bf16, kind="ExternalInput")
    # layer-0 gather tables + slot-major xw stage are host-computed
    xwA0_in = nc.dram_tensor("xwA0", [ROWS_A, 128], bf16,
                             kind="ExternalInput")
    xwB0_in = nc.dram_tensor("xwB0", [ROWS_B, 128], bf16,
                             kind="ExternalInput")
    xws0_in = nc.dram_tensor("xws0", [128, NW * 64], bf16,
                             kind="ExternalInput")
    src_idx = nc.dram_tensor("src_idx", [128, Tf * 8], i16,
                             kind="ExternalInput")
    vm_in = nc.dram_tensor("vm8", [128, Tf * 128], fp8, kind="ExternalInput")
    src_idx2 = nc.dram_tensor("src_idx2", [128, T2 * 8], i16,
                              kind="ExternalInput")
    vm_in2 = nc.dram_tensor("vm8_2", [128, T2 * 128], fp8,
                            kind="ExternalInput")
    dinv_in = nc.dram_tensor("dinv_own", [128, NW], f32, kind="ExternalInput")
    sqdeg_in = nc.dram_tensor("sqdeg_own", [1, NPC], bf16, kind="ExternalInput")
    Wmat = nc.dram_tensor("Wmat", [3, 64, 64], bf16, kind="ExternalInput")
    bias_in = nc.dram_tensor("bias_r", [3, 1, 64], bf16, kind="ExternalInput")
    ident_in = nc.dram_tensor("ident", [128, 128], bf16, kind="ExternalInput")
    agents = nc.dram_tensor("agents_out", [3, NW * 32, 64], bf16,
                            kind="ExternalOutput")

    with tile.TileContext(nc) as tc:
        with (
            tc.tile_pool(name="const", bufs=1) as constp,
            tc.tile_pool(name="hT", bufs=2) as hTp,
            tc.tile_pool(name="xws", bufs=2) as xwsp,
            tc.tile_pool(name="acc", bufs=2) as accp,
            tc.tile_pool(name="msg", bufs=6) as msgp,
            tc.tile_pool(name="vm", bufs=6) as vmp,
            tc.tile_pool(name="small", bufs=4) as smallp,
            tc.tile_pool(name="ps_seg", bufs=3, space="PSUM") as ps_seg,
            tc.tile_pool(name="ps_xw", bufs=2, space="PSUM") as ps_xw,
            tc.tile_pool(name="ps_tr", bufs=2, space="PSUM") as ps_tr,
            tc.tile_pool(name="dram_ag", bufs=2, space="DRAM") as dram_ag,
            tc.tile_pool(name="dram_xw", bufs=2, space="DRAM") as dram_xw,
        ):
            meta_idx_f = constp.tile([128, Tf * 8], i16, name="mi_f")
            nc.sync.dma_start(out=meta_idx_f[:], in_=src_idx[:, :])
            meta_idx_2 = constp.tile([128, T2 * 8], i16, name="mi_2")
            nc.sync.dma_start(out=meta_idx_2[:], in_=src_idx2[:, :])
            dinv_t = constp.tile([128, NW], f32)
            sqdeg_t = constp.tile([1, NPC], bf16)
            ident_t = constp.tile([128, 128], bf16)
            nc.sync.dma_start(out=dinv_t[:], in_=dinv_in[:, :])
            nc.sync.dma_start(out=sqdeg_t[:], in_=sqdeg_in[:, :])
            nc.sync.dma_start(out=ident_t[:], in_=ident_in[:, :])
            w_tiles = []
            b_tiles = []
            for l in range(3):
                wt = constp.tile([64, 64], bf16, name=f"w{l}")
                bt = constp.tile([1, 64], bf16, name=f"b{l}")
                nc.sync.dma_start(out=wt[:], in_=Wmat[l, :, :])
                nc.sync.dma_start(out=bt[:], in_=bias_in[l, :, :])
                w_tiles.append(wt)
                b_tiles.append(bt)

            hT_cur = hTp.tile([64, NPC], bf16, tag="hT", name="hT0")
            nc.sync.dma_start(out=hT_cur[:], in_=xT_own[:, :])

            gg = 0      # global gather counter: Tile assigns SWDGE sems as
                        # gather#%8, so queue gather#%4 keeps sem<->queue 1:1
            NWLO = HALF_A // WSZ        # 25 windows feed table A
            hTs = {0: hT_cur}
            xws, ags, xwAs, xwBs = {}, {}, {}, {}

            def make_xw_tiles(l):
                xws[l] = xwsp.tile([128, NW * 64], bf16, tag="xws",
                                   name=f"xws{l}")
                ags[l] = dram_ag.tile([NPC, 128], bf16, tag="ag",
                                      name=f"ag{l}")
                xwAs[l] = dram_xw.tile([ROWS_A, 128], bf16, tag="xwA",
                                       addr_space="Shared", name=f"xwA{l}")
                xwBs[l] = dram_xw.tile([ROWS_B, 128], bf16, tag="xwB",
                                       addr_space="Shared", name=f"xwB{l}")

            def emit_xw_win(l, w):
                """One window of xw(l) = (h_l @ W_l) * dinv, evacuated to the
                slot-major stage AND stored to this core's slice of the AG
                input buffer. Emitted inside the producing epilogue so every
                AllGather trigger finds its input already in DRAM."""
                ps = ps_xw.tile([128, 64], f32, tag="psxw",
                                name=f"psxw{l}_{w}")
                nc.tensor.matmul(
                    out=ps[:],
                    lhsT=hTs[l][:, w * 128:(w + 1) * 128],
                    rhs=w_tiles[l][:],
                    start=True, stop=True,
                )
                nc.scalar.mul(out=xws[l][:, w * 64:(w + 1) * 64],
                              in_=ps[:], mul=dinv_t[:, w:w + 1])
                nc.sync.dma_start(
                    out=ags[l][w * 128:(w + 1) * 128, 0:64],
                    in_=xws[l][:, w * 64:(w + 1) * 64],
                )

            def trigger_ag(l, lo):
                r0, r1 = (0, HALF_A) if lo else (HALF_A, NPC)
                dst = xwAs[l] if lo else xwBs[l]
                if "nocoll" in VAR:
                    nc.sync.dma_start(out=dst[0:r1 - r0, :],
                                      in_=ags[l][r0:r1, :])
                else:
                    BassGpSimd.collective_compute(
                        nc.gpsimd,
                        "AllGather",
                        mybir.AluOpType.bypass,
                        replica_groups=[list(range(N_CORES))],
                        ins=[ags[l][r0:r1, :].opt()],
                        outs=[dst.opt()],
                    )

            for l in range(3):
                if l == 0:
                    # host-computed layer-0 tables: no AG, just load xws
                    xws[0] = xwsp.tile([128, NW * 64], bf16, tag="xws",
                                       name="xws0")
                    nc.sync.dma_start(out=xws[0][:], in_=xws0_in[:, :])
                    xwA_ap = xwA0_in[:, :]
                    xwB_ap = xwB0_in[:, :]
                else:
                    xwA_ap = xwAs[l][:]
                    xwB_ap = xwBs[l][:]
                xw_stage = xws[l]

                if l < 2:
                    hT_next = hTp.tile([64, NPC], bf16, tag="hT",
                                       name=f"hT{l + 1}")
                    hTs[l + 1] = hT_next
                    make_xw_tiles(l + 1)
                else:
                    hT_next = None

                # parked phase-A partial sums, one [128, 64] slice per window
                acc_t = accp.tile([128, NW * 64], bf16, tag="acc",
                                  name=f"acc{l}")

                def epilogue(w, cur_ps):
                    hwin = smallp.tile([128, 64], bf16, tag="hwin",
                                       name=f"hw{l}_{w}")
                    nc.scalar.activation(
                        out=hwin[:], in_=cur_ps[:],
                        func=mybir.ActivationFunctionType.Sigmoid,
                        scale=dinv_t[:, w:w + 1],
                    )
                    nc.sync.dma_start(
                        out=agents[l, w * 32:(w + 1) * 32, :],
                        in_=hwin[0:128:4, :],
                    )
                    if hT_next is not None:
                        pt = ps_tr.tile([64, 128], bf16, tag="tr",
                                        name=f"tr{l}_{w}")
                        nc.tensor.transpose(out=pt[:], in_=hwin[:],
                                            identity=ident_t[:])
                        nc.scalar.copy(
                            out=hT_next[:, w * 128:(w + 1) * 128],
                            in_=pt[:],
                        )
                        # next layer's xw for this window, staged + stored
                        # so the AG triggers never wait on a compute chain
                        emit_xw_win(l + 1, w)

                # ---- gather + two-phase windowed segment-sum ----
                # layer 2 uses the agent-dst-only schedule
                S = scheds[l]
                runs = S["runs"]
                tile_win = S["tile_win"]
                n_th = S["n_th"]
                wfirst = S["wfirst"]
                wlast = S["wlast"]
                meta_idx = meta_idx_f if l < 2 else meta_idx_2
                vm_l = vm_in if l < 2 else vm_in2
                # xw-hi + AG_b(l) are emitted right after the first phase-A
                # run: the collective's input only needs hT(l) (ready at layer
                # start), so AG_b executes concurrently with the whole phase-A
                # gather stream instead of serializing before phase B.
                # AG_a(l+1) is hoisted to just after the run containing the
                # last lo-window epilogue, overlapping the phase-B tail.
                lo_last = max(
                    (wlast[1, w] if wlast[1, w] >= 0 else wlast[0, w])
                    for w in range(NWLO))
                win_ps = {}
                hi_done = (l == 0)
                hookA_done = (l >= 2)
                for r, (t0, nt, h) in enumerate(runs):
                    if r == 1 and not hi_done:
                        # early: AG_b(l) overlaps the phase-A gather stream;
                        # its input was stored during layer l-1's epilogues
                        trigger_ag(l, lo=False)
                        hi_done = True
                    msg = msgp.tile([128, max_run, 128], bf16, tag="msg",
                                    name=f"msg{l}_{r}")
                    vm_t = vmp.tile([128, max_run * 128], fp8, tag="vm",
                                    name=f"vm{l}_{r}")
                    nc.sync.dma_start(
                        out=vm_t[:, :nt * 128],
                        in_=vm_l[:, t0 * 128:(t0 + nt) * 128])
                    if "nogather" not in VAR:
                        # single_packet amortizes SDMA per-packet overhead but
                        # caps a call at 64 descs/engine = 8 tiles; sub-gathers
                        # share the run's queue so buffer/queue sems align
                        sp = "nosp" not in VAR
                        step = 8 if sp else nt
                        for s0 in range(0, nt, step):
                            sn = min(step, nt - s0)
                            nc.gpsimd.dma_gather(
                                out_ap=msg[:, s0:s0 + sn, :],
                                in_ap=(xwA_ap if h == 0 else xwB_ap),
                                idxs_ap=meta_idx[:, (t0 + s0) * 8:
                                                 (t0 + s0 + sn) * 8],
                                num_idxs=sn * 128,
                                num_idxs_reg=sn * 128,
                                elem_size=128,
                                single_packet=sp,
                                queue_num=(gg % nsq),
                            )
                            gg += 1
                    for j in range(nt):
                        t = t0 + j
                        w = int(tile_win[t])
                        if t == wfirst[h, w]:
                            cur = ps_seg.tile([128, 64], f32, tag="seg",
                                              name=f"seg{l}_{h}_{w}")
                            win_ps[w] = cur
                            # self-loop xw[w]: hi-half xws exists from run 1
                            # on; window 25+'s first tile is always later
                            # (phase A has >= 2 runs), lo windows use xws-lo
                            loop_here = (h == 0 or wfirst[0, w] < 0)
                            if h == 0 or wfirst[0, w] < 0:
                                # open with bias: psum = sqrt(deg) x bias
                                nc.tensor.matmul(
                                    out=cur[:],
                                    lhsT=sqdeg_t[:, w * 128:(w + 1) * 128],
                                    rhs=b_tiles[l][:],
                                    start=True, stop=False,
                                )
                            else:
                                # re-inject parked phase-A sum
                                nc.tensor.matmul(
                                    out=cur[:],
                                    lhsT=ident_t[:],
                                    rhs=acc_t[:, w * 64:(w + 1) * 64],
                                    start=True, stop=False,
                                )
                            if loop_here:
                                # self-loop term: += dinv[i] * xw[i]
                                nc.tensor.matmul(
                                    out=cur[:],
                                    lhsT=ident_t[:],
                                    rhs=xw_stage[:, w * 64:(w + 1) * 64],
                                    start=False, stop=False,
                                )
                        cur_ps = win_ps[w]
                        last = (t == wlast[h, w])
                        if "nomm" not in VAR:
                            nc.tensor.matmul(
                                out=cur_ps[:],
                                lhsT=vm_t[:, j * 128:(j + 1) * 128],
                                rhs=msg[:, j, 0:64],
                                start=False, stop=last,
                            )
                        elif last:
                            nc.scalar.copy(out=cur_ps[:], in_=cur_ps[:])
                        if last:
                            if h == 0 and wlast[1, w] >= 0:
                                # park phase-A sum in SBUF
                                nc.scalar.copy(
                                    out=acc_t[:, w * 64:(w + 1) * 64],
                                    in_=cur_ps[:])
                            else:
                                epilogue(w, cur_ps)
                    if not hookA_done and t0 + nt - 1 >= lo_last:
                        # lo windows of xw(l+1) stored: AG_a(l+1) overlaps
                        # the remaining phase-B runs + epilogue tail
                        trigger_ag(l + 1, lo=True)
                        hookA_done = True

                # windows with no tiles at all (pad safety)
                for w in range(NW):
                    if wlast[0, w] < 0 and wlast[1, w] < 0:
                        cur = ps_seg.tile([128, 64], f32, tag="seg",
                                          name=f"segz{l}_{w}")
                        nc.tensor.matmul(
                            out=cur[:],
                            lhsT=sqdeg_t[:, w * 128:(w + 1) * 128],
                            rhs=b_tiles[l][:],
                            start=True, stop=False,
                        )
                        nc.tensor.matmul(
                            out=cur[:],
                            lhsT=ident_t[:],
                            rhs=xw_stage[:, w * 64:(w + 1) * 64],
                            start=False, stop=True,
                        )
                        epilogue(w, cur)

                if l < 2 and not hookA_done:
                    trigger_ag(l + 1, lo=True)

                hT_cur = hT_next

    nc.compile()
    return nc


def kernel(**inputs):
    from concourse import bass_utils

    x = np.asarray(inputs["x"], dtype=np.float32)
    edge_index = np.asarray(inputs["edge_index"])
    agent_idx = np.asarray(inputs["agent_idx"], dtype=np.int64)
    Ws = [np.asarray(inputs[f"W{i}"], dtype=np.float32) for i in range(3)]
    bs = [np.asarray(inputs[f"b{i}"], dtype=np.float32) for i in range(3)]

    (idx_f, vm_f, sched_f), (idx_2, vm_2, sched_2), dinv_own, sqdeg_own, \
        dinv_pad = _preprocess(edge_index)

    nc = _build_program(sched_f, sched_2)

    xpad = np.zeros((N_CORES * NPC, D), np.float32)
    xpad[:N_NODES] = x

    # layer-0 gather tables on the host: xw0 = (x @ W0) * dinv[src]
    xw0 = ((xpad.astype(ml_dtypes.bfloat16).astype(np.float32)
            @ np.asarray(Ws[0], np.float32).astype(ml_dtypes.bfloat16)
            .astype(np.float32))
           * dinv_pad[:, None]).astype(ml_dtypes.bfloat16)
    r = np.arange(ROWS_A)
    nodesA = (r // HALF_A) * NPC + (r % HALF_A)
    r = np.arange(ROWS_B)
    nodesB = (r // HALF_B) * NPC + HALF_A + (r % HALF_B)
    xwA0 = np.zeros((ROWS_A, 128), ml_dtypes.bfloat16)
    xwA0[:, 0:64] = xw0[nodesA]
    xwB0 = np.zeros((ROWS_B, 128), ml_dtypes.bfloat16)
    xwB0[:, 0:64] = xw0[nodesB]
    xws0 = np.stack([
        xw0[c * NPC:(c + 1) * NPC].reshape(NW, WSZ, D)
        .transpose(1, 0, 2).reshape(WSZ, NW * D)
        for c in range(N_CORES)])
    Wstack = np.ascontiguousarray(
        np.stack(Ws)).astype(ml_dtypes.bfloat16)
    bias_stack = np.ascontiguousarray(
        np.stack([b[None, :] for b in bs])).astype(ml_dtypes.bfloat16)
    ident = np.eye(128, dtype=ml_dtypes.bfloat16)

    in_maps = []
    for c in range(N_CORES):
        in_maps.append({
            "xT_own": np.ascontiguousarray(
                xpad[c * NPC:(c + 1) * NPC].T).astype(ml_dtypes.bfloat16),
            "src_idx": np.ascontiguousarray(idx_f[c]),
            "vm8": np.ascontiguousarray(vm_f[c]),
            "src_idx2": np.ascontiguousarray(idx_2[c]),
            "vm8_2": np.ascontiguousarray(vm_2[c]),
            "xwA0": xwA0,
            "xwB0": xwB0,
            "xws0": np.ascontiguousarray(xws0[c]),
            "dinv_own": np.ascontiguousarray(dinv_own[c]),
            "sqdeg_own": np.ascontiguousarray(sqdeg_own[c]),
            "Wmat": Wstack,
            "bias_r": bias_stack,
            "ident": ident,
        })

    res = bass_utils.run_bass_kernel_spmd(
        nc, in_maps, core_ids=list(range(N_CORES)))

    taps = np.stack([np.asarray(res.results[c]["agents_out"])
                     .astype(np.float32) for c in range(N_CORES)])
    # taps[c, l, r, :] = h_l for node (c*NPC + 4*r)
    n_agents = agent_idx.shape[0]
    out = np.empty((n_agents, 3 * D), np.float32)
    c_of = agent_idx // NPC
    r_of = (agent_idx % NPC) // 4
    for l in range(3):
        out[:, l * D:(l + 1) * D] = taps[c_of, l, r_of, :]
    return out



# revision 39
# speedup vs baseline: 1.0080x; 1.0080x over previous
"""3-layer GCN (PyG GCNConv-style) on 8 Trainium2 NeuronCores — v3.

Strategy (graph/data parallel; nodes sharded by destination core):
  - Nodes partitioned contiguously: 6272 per core (49 windows x 128). Edges
    (incl. host-added self-loops) are owned by the core owning their dst.
  - Normalization is folded away: the gathered xw table holds
    dinv[src] * (h @ W) rows, and the window epilogue applies the dinv[dst]
    factor as the per-partition `scale` of the sigmoid activation. The bias
    is injected into PSUM via a K=1 rank-1 matmul with a sqrt(deg[dst])
    column so it survives the later dinv[dst] scaling.
  - The per-edge one-hot scatter matrices are graph-static: built ONCE on
    the host in fp8 (exact for 0/1) and streamed from DRAM each layer.
  - Gathers use int16 indices, so the 50176-row xw table is split into
    half-tables A (local row < 3200, 25600 rows) and B (24576 rows). Each
    layer runs two phases: phase A processes every window's A-half tiles
    (window-major, parked to SBUF in bf16 at window close), phase B
    re-injects the parked sum via an identity matmul and finishes the
    window (sigmoid epilogue, agent tap, PE transpose into the next
    layer's hT). Phases are split into CMAX-tile gather chunks, each
    issued as 8-tile sub-gathers (single_packet coalescing caps a call at
    64 descs/engine) spread round-robin over the 4 SWDGE queues; the
    steady state is bound by SDMA descriptor execution (~45ns per 256B
    descriptor per engine across 16 engines).
  - v3 over v2:
    * Layer 2 uses a second, smaller edge schedule keeping only edges into
      tapped destinations (agents, dst%4==0): 875 -> 294 gather tiles.
    * Layer 0's gather tables and slot-major xw stage are computed on the
      HOST (x and W0 are known), removing both layer-0 AllGathers and the
      ~110us serial startup they caused.
    * AllGather DRAM buffers ping-pong (bufs=2) so a layer's collective
      never waits for the previous layer's gather reads to drain (this
      WAR stall exposed ~120us per layer on the tail).
    * AG_b(l) is emitted after the first phase-A run (its input only needs
      hT(l), ready at layer start) so it overlaps the whole phase-A gather
      stream; AG_a(l+1) is emitted right after the run containing the last
      lo-window epilogue, overlapping the phase-B tail.

Host-side work: graph preprocessing (degrees, edge layout, one-hot tiles,
layer-0 tables) and final output assembly.
"""

import sys

sys.path.insert(0, "/opt/trn_rl_repo")

import numpy as np
import ml_dtypes

N_NODES = 50000
D = 64
N_CORES = 8
WSZ = 128               # dst-window size (PSUM partition dim)
NW = 49                 # windows per core
NPC = NW * WSZ          # 6272 padded nodes per core (50176 total >= 50000)
HALF_A = 3200           # local rows < HALF_A -> table A (25 windows' rows)
HALF_B = NPC - HALF_A   # 3072 rows -> table B
ROWS_A = N_CORES * HALF_A   # 25600 (< 32767, int16-addressable)
ROWS_B = N_CORES * HALF_B   # 24576
CMAX = 48               # max tiles per gather chunk; small chunks + deep
                        # buffering let 4 queues (= Q7 core pairs) gen in parallel


def _layout(s_all, d_all):
    """Edge layout + one-hot scatter tiles for one edge set.

    Tile stream: [phase A: w0..w48, each window's A-half tiles]
                 [phase B: w0..w48, each window's B-half tiles].
    SPMD: tile counts per (window, half) are maxed over cores; padded slots
    get all-zero one-hot rows so they contribute nothing.
    """
    core = d_all // NPC
    local = d_all - core * NPC
    win = local // WSZ
    col = local % WSZ

    s_core = s_all // NPC
    s_loc = s_all - s_core * NPC
    half = (s_loc >= HALF_A).astype(np.int64)
    idx16 = np.where(half == 0, s_core * HALF_A + s_loc,
                     s_core * HALF_B + (s_loc - HALF_A))

    # group edges by (core, half, win)
    key = (core * 2 + half) * NW + win
    nkey = N_CORES * 2 * NW
    order = np.argsort(key, kind="stable")
    key_sorted = key[order]
    bounds = np.searchsorted(key_sorted, np.arange(nkey + 1))
    cnt = (bounds[1:] - bounds[:-1]).reshape(N_CORES, 2, NW)

    # uniform tiles per (half, win), maxed over cores
    n_th = -(-cnt.max(axis=0) // WSZ)               # [2, NW]
    T = int(n_th.sum())

    # tile stream + gather chunks (runs)
    tile_win = []
    runs = []                                       # (t0, nt, half)
    win_tile_base = np.zeros((2, NW), np.int64)
    for h in (0, 1):
        p0 = len(tile_win)
        for w in range(NW):
            win_tile_base[h, w] = len(tile_win)
            tile_win += [w] * int(n_th[h, w])
        np_h = len(tile_win) - p0                   # tiles in this phase
        if np_h == 0:
            continue
        n_chunks = -(-np_h // CMAX)
        splits = np.linspace(p0, p0 + np_h, n_chunks + 1).astype(np.int64)
        for a, b in zip(splits[:-1], splits[1:]):
            if b > a:
                runs.append((int(a), int(b - a), h))
    tile_win = np.asarray(tile_win)
    assert len(tile_win) == T
    max_run = max(nt for _, nt, _ in runs)

    # per-window first/last tile within each phase (-1 if none)
    wfirst = np.full((2, NW), -1, np.int64)
    wlast = np.full((2, NW), -1, np.int64)
    for h in (0, 1):
        for w in range(NW):
            if n_th[h, w] > 0:
                wfirst[h, w] = win_tile_base[h, w]
                wlast[h, w] = win_tile_base[h, w] + n_th[h, w] - 1

    # per-core edge slot arrays
    idx_flat = np.zeros((N_CORES, T * WSZ), np.int16)
    vm8 = np.zeros((N_CORES, WSZ, T * WSZ), ml_dtypes.float8_e4m3)
    for c in range(N_CORES):
        for h in (0, 1):
            for w in range(NW):
                gidx = (c * 2 + h) * NW + w
                e0, e1 = bounds[gidx], bounds[gidx + 1]
                n = e1 - e0
                if n == 0:
                    continue
                sel = order[e0:e1]
                base = win_tile_base[h, w] * WSZ
                pos = base + np.arange(n)
                idx_flat[c, pos] = idx16[sel].astype(np.int16)
                tt = pos // WSZ
                pp = pos % WSZ
                vm8[c, pp, tt * WSZ + col[sel]] = 1.0

    # wrap indices for dma_gather: [128, T*8] int16,
    # arr[p, t*8 + cc] = idx[t*128 + cc*16 + (p % 16)]
    w16 = idx_flat.reshape(N_CORES, T, 8, 16).transpose(0, 3, 1, 2).reshape(
        N_CORES, 16, T * 8)
    idx_arr = np.tile(w16, (1, 8, 1))               # [N_CORES, 128, T*8]

    sched = dict(T=T, runs=runs, tile_win=tile_win, n_th=n_th,
                 wfirst=wfirst, wlast=wlast, max_run=max_run)
    return idx_arr, vm8, sched


def _preprocess(edge_index):
    src = np.asarray(edge_index[0], dtype=np.int64)
    dst = np.asarray(edge_index[1], dtype=np.int64)

    deg = np.bincount(dst, minlength=N_NODES).astype(np.float32) + 1.0
    dinv = (1.0 / np.sqrt(deg)).astype(np.float32)
    sqdeg = np.sqrt(deg).astype(np.float32)

    # full edge set (layers 0, 1); self-loops are NOT gathered: their
    # dinv[i]*xw[i] rows live in the local xw_stage (identity matmul)
    idx_f, vm_f, sched_f = _layout(src, dst)
    # layer 2 only needs destinations that are tapped: agent rows (dst%4==0)
    m2 = (dst % 4) == 0
    idx_2, vm_2, sched_2 = _layout(src[m2], dst[m2])

    dinv_pad = np.ones(N_CORES * NPC, np.float32)
    sqdeg_pad = np.ones(N_CORES * NPC, np.float32)
    dinv_pad[:N_NODES] = dinv
    sqdeg_pad[:N_NODES] = sqdeg
    dinv_own = dinv_pad.reshape(N_CORES, NW, WSZ).transpose(0, 2, 1).copy()
    sqdeg_own = sqdeg_pad.reshape(N_CORES, 1, NPC).astype(ml_dtypes.bfloat16)

    return (idx_f, vm_f, sched_f), (idx_2, vm_2, sched_2), \
        dinv_own, sqdeg_own, dinv_pad


def _build_program(sched_f, sched_2):
    import os
    VAR = set(os.environ.get("KVAR", "").split(","))
    import concourse.bass as bass
    import concourse.bacc as bacc
    import concourse.tile as tile
    from concourse import mybir
    from concourse.bass import BassGpSimd

    f32 = mybir.dt.float32
    bf16 = mybir.dt.bfloat16
    fp8 = mybir.dt.float8e4
    i16 = mybir.dt.int16

    scheds = [sched_f, sched_f, sched_2]
    Tf = sched_f["T"]
    T2 = sched_2["T"]
    max_run = max(sched_f["max_run"], sched_2["max_run"])

    nsq = 4
    nc = bacc.Bacc("TRN2", target_bir_lowering=False, debug=False,
                   num_devices=N_CORES, num_swdge_queues=nsq)

    xT_own = nc.dram_tensor("xT_own", [64, NPC], bf16, kind="ExternalInput")
    # layer-0 gather tables + slot-major xw stage are host-computed
    xwA0_in = nc.dram_tensor("xwA0", [ROWS_A, 128], bf16,
                             kind="ExternalInput")
    xwB0_in = nc.dram_tensor("xwB0", [ROWS_B, 128], bf16,
                             kind="ExternalInput")
    xws0_in = nc.dram_tensor("xws0", [128, NW * 64], bf16,
                             kind="ExternalInput")
    src_idx = nc.dram_tensor("src_idx", [128, Tf * 8], i16,
                             kind="ExternalInput")
    vm_in = nc.dram_tensor("vm8", [128, Tf * 128], fp8, kind="ExternalInput")
    src_idx2 = nc.dram_tensor("src_idx2", [128, T2 * 8], i16,
                              kind="ExternalInput")
    vm_in2 = nc.dram_tensor("vm8_2", [128, T2 * 128], fp8,
                            kind="ExternalInput")
    dinv_in = nc.dram_tensor("dinv_own", [128, NW], f32, kind="ExternalInput")
    sqdeg_in = nc.dram_tensor("sqdeg_own", [1, NPC], bf16, kind="ExternalInput")
    Wmat = nc.dram_tensor("Wmat", [3, 64, 64], bf16, kind="ExternalInput")
    bias_in = nc.dram_tensor("bias_r", [3, 1, 64], bf16, kind="ExternalInput")
    ident_in = nc.dram_tensor("ident", [128, 128], bf16, kind="ExternalInput")
    agents = nc.dram_tensor("agents_out", [3, NW * 32, 64], bf16,
                            kind="ExternalOutput")

    with tile.TileContext(nc) as tc:
        with (
            tc.tile_pool(name="const", bufs=1) as constp,
            tc.tile_pool(name="hT", bufs=2) as hTp,
            tc.tile_pool(name="xws", bufs=2) as xwsp,
            tc.tile_pool(name="acc", bufs=2) as accp,
            tc.tile_pool(name="msg", bufs=6) as msgp,
            tc.tile_pool(name="vm", bufs=6) as vmp,
            tc.tile_pool(name="small", bufs=4) as smallp,
            tc.tile_pool(name="ps_seg", bufs=3, space="PSUM") as ps_seg,
            tc.tile_pool(name="ps_xw", bufs=2, space="PSUM") as ps_xw,
            tc.tile_pool(name="ps_tr", bufs=2, space="PSUM") as ps_tr,
            tc.tile_pool(name="dram_ag", bufs=2, space="DRAM") as dram_ag,
            tc.tile_pool(name="dram_xw", bufs=2, space="DRAM") as dram_xw,
        ):
            meta_idx_f = constp.tile([128, Tf * 8], i16, name="mi_f")
            nc.sync.dma_start(out=meta_idx_f[:], in_=src_idx[:, :])
            meta_idx_2 = constp.tile([128, T2 * 8], i16, name="mi_2")
            nc.sync.dma_start(out=meta_idx_2[:], in_=src_idx2[:, :])
            dinv_t = constp.tile([128, NW], f32)
            sqdeg_t = constp.tile([1, NPC], bf16)
            ident_t = constp.tile([128, 128], bf16)
            nc.sync.dma_start(out=dinv_t[:], in_=dinv_in[:, :])
            nc.sync.dma_start(out=sqdeg_t[:], in_=sqdeg_in[:, :])
            nc.sync.dma_start(out=ident_t[:], in_=ident_in[:, :])
            w_tiles = []
            b_tiles = []
            for l in range(3):
                wt = constp.tile([64, 64], bf16, name=f"w{l}")
                bt = constp.tile([1, 64], bf16, name=f"b{l}")
                nc.sync.dma_start(out=wt[:], in_=Wmat[l, :, :])
                nc.sync.dma_start(out=bt[:], in_=bias_in[l, :, :])
                w_tiles.append(wt)
                b_tiles.append(bt)

            hT_cur = hTp.tile([64, NPC], bf16, tag="hT", name="hT0")
            nc.sync.dma_start(out=hT_cur[:], in_=xT_own[:, :])

            gg = 0      # global gather counter: Tile assigns SWDGE sems as
                        # gather#%8, so queue gather#%4 keeps sem<->queue 1:1
            NWLO = HALF_A // WSZ        # 25 windows feed table A
            hTs = {0: hT_cur}
            xws, ags, xwAs, xwBs = {}, {}, {}, {}

            def emit_xw_half(l, lo):
                """xw = (h @ W_l) * dinv for windows [0,25) or [25,49), plus
                the matching shard store + AllGather. The lo half of layer
                l+1 is emitted inside layer l so AG_a overlaps its PE tail."""
                if l not in xws:
                    xws[l] = xwsp.tile([128, NW * 64], bf16, tag="xws",
                                       name=f"xws{l}")
                    ags[l] = dram_ag.tile([NPC, 128], bf16, tag="ag",
                                          name=f"ag{l}")
                    xwAs[l] = dram_xw.tile([ROWS_A, 128], bf16, tag="xwA",
                                           addr_space="Shared",
                                           name=f"xwA{l}")
                    xwBs[l] = dram_xw.tile([ROWS_B, 128], bf16, tag="xwB",
                                           addr_space="Shared",
                                           name=f"xwB{l}")
                ws = range(0, NWLO) if lo else range(NWLO, NW)
                for w in ws:
                    ps = ps_xw.tile([128, 64], f32, tag="psxw",
                                    name=f"psxw{l}_{w}")
                    nc.tensor.matmul(
                        out=ps[:],
                        lhsT=hTs[l][:, w * 128:(w + 1) * 128],
                        rhs=w_tiles[l][:],
                        start=True, stop=True,
                    )
                    nc.scalar.mul(out=xws[l][:, w * 64:(w + 1) * 64],
                                  in_=ps[:], mul=dinv_t[:, w:w + 1])
                r0, r1 = (0, HALF_A) if lo else (HALF_A, NPC)
                nc.sync.dma_start(
                    out=ags[l][r0:r1, :].rearrange(
                        "(w p) f -> p w f", p=128)[:, :, 0:64],
                    in_=xws[l][:, r0 // 2:r1 // 2].rearrange(
                        "p (w f) -> p w f", f=64),
                )
                dst = xwAs[l] if lo else xwBs[l]
                if "nocoll" in VAR:
                    nc.sync.dma_start(out=dst[0:r1 - r0, :],
                                      in_=ags[l][r0:r1, :])
                else:
                    BassGpSimd.collective_compute(
                        nc.gpsimd,
                        "AllGather",
                        mybir.AluOpType.bypass,
                        replica_groups=[list(range(N_CORES))],
                        ins=[ags[l][r0:r1, :].opt()],
                        outs=[dst.opt()],
                    )

            for l in range(3):
                if l == 0:
                    # host-computed layer-0 tables: no AG, just load xws
                    xws[0] = xwsp.tile([128, NW * 64], bf16, tag="xws",
                                       name="xws0")
                    nc.sync.dma_start(out=xws[0][:], in_=xws0_in[:, :])
                    xwA_ap = xwA0_in[:, :]
                    xwB_ap = xwB0_in[:, :]
                else:
                    xwA_ap = xwAs[l][:]
                    xwB_ap = xwBs[l][:]
                xw_stage = xws[l]

                if l < 2:
                    hT_next = hTp.tile([64, NPC], bf16, tag="hT",
                                       name=f"hT{l + 1}")
                    hTs[l + 1] = hT_next
                else:
                    hT_next = None

                # parked phase-A partial sums, one [128, 64] slice per window
                acc_t = accp.tile([128, NW * 64], bf16, tag="acc",
                                  name=f"acc{l}")

                def epilogue(w, cur_ps):
                    hwin = smallp.tile([128, 64], bf16, tag="hwin",
                                       name=f"hw{l}_{w}")
                    nc.scalar.activation(
                        out=hwin[:], in_=cur_ps[:],
                        func=mybir.ActivationFunctionType.Sigmoid,
                        scale=dinv_t[:, w:w + 1],
                    )
                    nc.sync.dma_start(
                        out=agents[l, w * 32:(w + 1) * 32, :],
                        in_=hwin[0:128:4, :],
                    )
                    if hT_next is not None:
                        pt = ps_tr.tile([64, 128], bf16, tag="tr",
                                        name=f"tr{l}_{w}")
                        nc.tensor.transpose(out=pt[:], in_=hwin[:],
                                            identity=ident_t[:])
                        nc.scalar.copy(
                            out=hT_next[:, w * 128:(w + 1) * 128],
                            in_=pt[:],
                        )

                # ---- gather + two-phase windowed segment-sum ----
                # layer 2 uses the agent-dst-only schedule
                S = scheds[l]
                runs = S["runs"]
                tile_win = S["tile_win"]
                n_th = S["n_th"]
                wfirst = S["wfirst"]
                wlast = S["wlast"]
                meta_idx = meta_idx_f if l < 2 else meta_idx_2
                vm_l = vm_in if l < 2 else vm_in2
                # xw-hi + AG_b(l) are emitted right after the first phase-A
                # run: the collective's input only needs hT(l) (ready at layer
                # start), so AG_b executes concurrently with the whole phase-A
                # gather stream instead of serializing before phase B.
                # AG_a(l+1) is hoisted to just after the run containing the
                # last lo-window epilogue, overlapping the phase-B tail.
                lo_last = max(
                    (wlast[1, w] if wlast[1, w] >= 0 else wlast[0, w])
                    for w in range(NWLO))
                win_ps = {}
                hi_done = (l == 0)
                hookA_done = (l >= 2)
                for r, (t0, nt, h) in enumerate(runs):
                    if r == 1 and not hi_done:
                        # early: AG_b(l) overlaps the phase-A gather stream
                        emit_xw_half(l, lo=False)
                        hi_done = True
                    msg = msgp.tile([128, max_run, 128], bf16, tag="msg",
                                    name=f"msg{l}_{r}")
                    vm_t = vmp.tile([128, max_run * 128], fp8, tag="vm",
                                    name=f"vm{l}_{r}")
                    nc.sync.dma_start(
                        out=vm_t[:, :nt * 128],
                        in_=vm_l[:, t0 * 128:(t0 + nt) * 128])
                    if "nogather" not in VAR:
                        # single_packet amortizes SDMA per-packet overhead but
                        # caps a call at 64 descs/engine = 8 tiles; sub-gathers
                        # share the run's queue so buffer/queue sems align
                        sp = "nosp" not in VAR
                        step = 8 if sp else nt
                        for s0 in range(0, nt, step):
                            sn = min(step, nt - s0)
                            nc.gpsimd.dma_gather(
                                out_ap=msg[:, s0:s0 + sn, :],
                                in_ap=(xwA_ap if h == 0 else xwB_ap),
                                idxs_ap=meta_idx[:, (t0 + s0) * 8:
                                                 (t0 + s0 + sn) * 8],
                                num_idxs=sn * 128,
                                num_idxs_reg=sn * 128,
                                elem_size=128,
                                single_packet=sp,
                                queue_num=(gg % nsq),
                            )
                            gg += 1
                    for j in range(nt):
                        t = t0 + j
                        w = int(tile_win[t])
                        if t == wfirst[h, w]:
                            cur = ps_seg.tile([128, 64], f32, tag="seg",
                                              name=f"seg{l}_{h}_{w}")
                            win_ps[w] = cur
                            # self-loop xw[w]: hi-half xws exists from run 1
                            # on; window 25+'s first tile is always later
                            # (phase A has >= 2 runs), lo windows use xws-lo
                            loop_here = (h == 0 or wfirst[0, w] < 0)
                            if h == 0 or wfirst[0, w] < 0:
                                # open with bias: psum = sqrt(deg) x bias
                                nc.tensor.matmul(
                                    out=cur[:],
                                    lhsT=sqdeg_t[:, w * 128:(w + 1) * 128],
                                    rhs=b_tiles[l][:],
                                    start=True, stop=False,
                                )
                            else:
                                # re-inject parked phase-A sum
                                nc.tensor.matmul(
                                    out=cur[:],
                                    lhsT=ident_t[:],
                                    rhs=acc_t[:, w * 64:(w + 1) * 64],
                                    start=True, stop=False,
                                )
                            if loop_here:
                                # self-loop term: += dinv[i] * xw[i]
                                nc.tensor.matmul(
                                    out=cur[:],
                                    lhsT=ident_t[:],
                                    rhs=xw_stage[:, w * 64:(w + 1) * 64],
                                    start=False, stop=False,
                                )
                        cur_ps = win_ps[w]
                        last = (t == wlast[h, w])
                        if "nomm" not in VAR:
                            nc.tensor.matmul(
                                out=cur_ps[:],
                                lhsT=vm_t[:, j * 128:(j + 1) * 128],
                                rhs=msg[:, j, 0:64],
                                start=False, stop=last,
                            )
                        elif last:
                            nc.scalar.copy(out=cur_ps[:], in_=cur_ps[:])
                        if last:
                            if h == 0 and wlast[1, w] >= 0:
                                # park phase-A sum in SBUF
                                nc.scalar.copy(
                                    out=acc_t[:, w * 64:(w + 1) * 64],
                                    in_=cur_ps[:])
                            else:
                                epilogue(w, cur_ps)
                    if not hookA_done and t0 + nt - 1 >= lo_last:
                        # lo windows of hT(l+1) final: AG_a(l+1) overlaps
                        # the remaining phase-B runs + epilogue tail
                        emit_xw_half(l + 1, lo=True)
                        hookA_done = True

                # windows with no tiles at all (pad safety)
                for w in range(NW):
                    if wlast[0, w] < 0 and wlast[1, w] < 0:
                        cur = ps_seg.tile([128, 64], f32, tag="seg",
                                          name=f"segz{l}_{w}")
                        nc.tensor.matmul(
                            out=cur[:],
                            lhsT=sqdeg_t[:, w * 128:(w + 1) * 128],
                            rhs=b_tiles[l][:],
                            start=True, stop=False,
                        )
                        nc.tensor.matmul(
                            out=cur[:],
                            lhsT=ident_t[:],
                            rhs=xw_stage[:, w * 64:(w + 1) * 64],
                            start=False, stop=True,
                        )
                        epilogue(w, cur)

                if l < 2 and not hookA_done:
                    emit_xw_half(l + 1, lo=True)

                hT_cur = hT_next

    nc.compile()
    return nc


def kernel(**inputs):
    from concourse import bass_utils

    x = np.asarray(inputs["x"], dtype=np.float32)
    edge_index = np.asarray(inputs["edge_index"])
    agent_idx = np.asarray(inputs["agent_idx"], dtype=np.int64)
    Ws = [np.asarray(inputs[f"W{i}"], dtype=np.float32) for i in range(3)]
    bs = [np.asarray(inputs[f"b{i}"], dtype=np.float32) for i in range(3)]

    (idx_f, vm_f, sched_f), (idx_2, vm_2, sched_2), dinv_own, sqdeg_own, \
        dinv_pad = _preprocess(edge_index)

    nc = _build_program(sched_f, sched_2)

    xpad = np.zeros((N_CORES * NPC, D), np.float32)
    xpad[:N_NODES] = x

    # layer-0 gather tables on the host: xw0 = (x @ W0) * dinv[src]
    xw0 = ((xpad.astype(ml_dtypes.bfloat16).astype(np.float32)
            @ np.asarray(Ws[0], np.float32).astype(ml_dtypes.bfloat16)
            .astype(np.float32))
           * dinv_pad[:, None]).astype(ml_dtypes.bfloat16)
    r = np.arange(ROWS_A)
    nodesA = (r // HALF_A) * NPC + (r % HALF_A)
    r = np.arange(ROWS_B)
    nodesB = (r // HALF_B) * NPC + HALF_A + (r % HALF_B)
    xwA0 = np.zeros((ROWS_A, 128), ml_dtypes.bfloat16)
    xwA0[:, 0:64] = xw0[nodesA]
    xwB0 = np.zeros((ROWS_B, 128), ml_dtypes.bfloat16)
    xwB0[:, 0:64] = xw0[nodesB]
    xws0 = np.stack([
        xw0[c * NPC:(c + 1) * NPC].reshape(NW, WSZ, D)
        .transpose(1, 0, 2).reshape(WSZ, NW * D)
        for c in range(N_CORES)])
    Wstack = np.ascontiguousarray(
        np.stack(Ws)).astype(ml_dtypes.bfloat16)
    bias_stack = np.ascontiguousarray(
        np.stack([b[None, :] for b in bs])).astype(ml_dtypes.bfloat16)
    ident = np.eye(128, dtype=ml_dtypes.bfloat16)

    in_maps = []
    for c in range(N_CORES):
        in_maps.append({
            "xT_own": np.ascontiguousarray(
                xpad[c * NPC:(c + 1) * NPC].T).astype(ml_dtypes.bfloat16),
            "src_idx": np.ascontiguousarray(idx_f[c]),
            "vm8": np.ascontiguousarray(vm_f[c]),
            "src_idx2": np.ascontiguousarray(idx_2[c]),
            "vm8_2": np.ascontiguousarray(vm_2[c]),
            "xwA0": xwA0,
            "xwB0": xwB0,
            "xws0": np.ascontiguousarray(xws0[c]),
            "dinv_own": np.ascontiguousarray(dinv_own[c]),
            "sqdeg_own": np.ascontiguousarray(sqdeg_own[c]),
            "Wmat": Wstack,
            "bias_r": bias_stack,
            "ident": ident,
        })

    res = bass_utils.run_bass_kernel_spmd(
        nc, in_maps, core_ids=list(range(N_CORES)))

    taps = np.stack([np.asarray(res.results[c]["agents_out"])
                     .astype(np.float32) for c in range(N_CORES)])
    # taps[c, l, r, :] = h_l for node (c*NPC + 4*r)
    n_agents = agent_idx.shape[0]
    out = np.empty((n_agents, 3 * D), np.float32)
    c_of = agent_idx // NPC
    r_of = (agent_idx % NPC) // 4
    for l in range(3):
        out[:, l * D:(l + 1) * D] = taps[c_of, l, r_of, :]
    return out

